# revision 18
# baseline (speedup 1.0000x reference)
"""GAT (2-layer, PyG GATConv heads=1) on 8 Trainium2 NeuronCores.

Strategy (self-contained; shapes hardcoded for the target problem):
 - dst-shard nodes across 8 cores; per layer run TWO passes (lo/hi half of
   the node table, because dma_gather indices are int16) with dst nodes
   degree-sorted into groups of 128.
 - slot-major edge layout: chunk j of a group = slot j across its 128 dst
   nodes, so partition index == dst index. Per-edge softmax scalars become
   per-partition ops and the scatter is an accumulating matmul with a
   constant identity lhsT into PSUM.
 - gather of source-node rows via dma_gather (4 SWDGE queues round-robin).
   Table row (512B, 256 bf16 slots): [ones(1) | xw_hi bf16 x128 |
   xw_lo bf16 x122 | es f32(2 slots) | ed f32(2 slots) | pad(1)].
 - lo-pass writes partial rows [den, msg(128), ed] (768B) to DRAM; hi-pass
   gathers them per-node, combines, normalizes, adds bias, selu.
 - x@W front-ends per core; AllGather of the 512B-row table between cores;
   mean-pool via host-built indicator matmul + AllReduce; FC head redundant.
"""

import math
import numpy as np
import ml_dtypes

import concourse.bacc as bacc
import concourse.bass as bass
import concourse.mybir as mybir
import concourse.tile as tile
from concourse.bass_utils import run_bass_kernel_spmd
from concourse.masks import make_identity

P = 128
NC = 8
F_IN = 768
F = 128
FLO = 122          # lo-precision features 0..121
NHID = 64
N_CLS = 2
N_GRAPHS = 128
SELU_L = 1.0507009873554805
SELU_A = 1.6732632423543772

f32 = mybir.dt.float32
bf16 = mybir.dt.bfloat16
fp16 = mybir.dt.float16
i16 = mybir.dt.int16

# row layout (bf16 slot indices)
C_ONE = 0
C_HI = 1           # cols 1..128
C_LO = 129         # cols 129..250  (features 0..121)
C_SCALE_END = 251  # scaled region = cols 0..250
C_ES = 252         # f32 at bf16 cols 252-253 -> f32 view col 126
C_ED = 254         # f32 view col 127
ROW = 256
PART_ROW = 192     # partial row: f32 [den, msg(128), ed, pad...] = 768B


class Cfg:
    def __init__(self, n_nodes, n_edges, groups_per_core, lo_lim, seed_pad=32):
        self.N = n_nodes
        self.E = n_edges
        self.G = groups_per_core
        self.NPC = groups_per_core * P
        self.SLAB = self.NPC + seed_pad
        self.V = NC * self.SLAB
        self.LO = lo_lim
        assert self.N % NC == 0
        assert self.NPC >= self.N // NC
        assert self.LO % 128 == 0 and self.LO < 32768 + 1
        # zero rows: core0 pad row (id NPC) for lo view; core7 pad for hi view
        self.ZLO = self.NPC
        self.ZHI = (NC - 1) * self.SLAB + self.NPC
        assert self.ZLO < self.LO
        assert self.ZHI >= self.LO and self.ZHI - self.LO < 32768
        assert self.V - self.LO < 32768


FULL = Cfg(50000, 800000, 49, 32768)


def _wrap(flat):
    """int16 index list [num] (num % 128 == 0) -> wrapped [128, num//16]."""
    num = len(flat)
    assert num % 128 == 0
    S = num // 16
    w = flat.reshape(S, 16).T.astype(np.int16)
    return np.tile(w, (8, 1))


def build_host(cfg, x, edge_index, batch):
    """Compute schedules (shared across cores) and per-core index tensors."""
    N, NPC, SLAB = cfg.N, cfg.NPC, cfg.SLAB
    src0 = edge_index[0].astype(np.int64)
    dst0 = edge_index[1].astype(np.int64)
    loops = np.arange(N, dtype=np.int64)
    src = np.concatenate([src0, loops])
    dst = np.concatenate([dst0, loops])

    deg = np.bincount(dst, minlength=N)
    order = np.argsort(-deg, kind="stable")
    node_core = np.empty(N, np.int32)
    node_core[order] = np.arange(N) % NC
    # initial (layer-1) position within core
    pos1 = np.empty(N, np.int64)
    for c in range(NC):
        mine = order[node_core[order] == c]
        pos1[mine] = np.arange(len(mine))
    tid1 = node_core * SLAB + pos1          # layer-1 table id per node

    meta = {"k": {}}                         # shared schedule
    per_core = [dict() for _ in range(NC)]

    # returns: per-core idx blob segments (built in the same order the kernel
    # builder walks), plus next-layer tid
    def do_layer(layer, tid):
        src_tid = tid[src]
        is_lo = src_tid < cfg.LO
        new_pos = np.empty(N, np.int64)       # this layer's hi-pass order
        for pas, sel in (("lo", is_lo), ("hi", ~is_lo)):
            s_src = src_tid[sel]
            s_dst = dst[sel]
            cdeg = np.bincount(s_dst, minlength=N)  # class degree per dst
            kmat = np.zeros((NC, cfg.G), np.int64)
            core_orders = []
            for c in range(NC):
                nodes_c = np.where(node_core == c)[0]
                # order by class-degree desc (stable); pad to NPC with -1
                o = nodes_c[np.argsort(-cdeg[nodes_c], kind="stable")]
                o = np.concatenate([o, -np.ones(NPC - len(o), np.int64)])
                core_orders.append(o)
                kc = cdeg[o[o >= 0]]
                kfull = np.zeros(NPC, np.int64)
                kfull[: len(kc)] = kc
                kmat[c] = kfull.reshape(cfg.G, P).max(1)
            khat = np.maximum(kmat.max(0), 1)
            meta["k"][(layer, pas)] = khat

            # per-core index arrays
            for c in range(NC):
                o = core_orders[c]
                # edges of this core&class: slot matrix [NPC, kmax]
                kmax = int(khat.max())
                zrow = cfg.ZLO if pas == "lo" else cfg.ZHI - cfg.LO
                M = np.full((NPC, kmax), zrow, np.int64)
                mask_c = node_core[s_dst] == c
                e_dst = s_dst[mask_c]
                e_src = s_src[mask_c]
                # position of dst in this pass order
                pord = np.empty(N, np.int64)
                real = o[o >= 0]
                pord[real] = np.arange(len(real))
                epos = pord[e_dst]
                so = np.argsort(epos, kind="stable")
                epos_s = epos[so]
                esrc_s = e_src[so]
                start = np.searchsorted(epos_s, np.arange(NPC))
                slot = np.arange(len(epos_s)) - start[epos_s]
                vals = esrc_s if pas == "lo" else esrc_s - cfg.LO
                M[epos_s, slot] = vals
                assert M.max() < 32768 and M.min() >= 0
                segs = per_core[c].setdefault(f"idx_l{layer}", [])
                for g in range(cfg.G):
                    rows = slice(g * P, (g + 1) * P)
                    if pas == "lo":
                        # ed-gather: local slab positions of group's dst
                        dpos = np.where(o[rows] >= 0,
                                        (tid[np.maximum(o[rows], 0)] % SLAB),
                                        cfg.NPC)  # dummies -> zero pad row
                        segs.append(_wrap(dpos.astype(np.int16)))
                    else:
                        # partial-gather: lo-pass position of group's dst
                        lp = per_core[c]["lo_pos"]
                        dpos = np.where(o[rows] >= 0,
                                        lp[np.maximum(o[rows], 0)],
                                        cfg.NPC)
                        segs.append(_wrap(dpos.astype(np.int16)))
                    kg = int(khat[g])
                    blk = M[rows, :kg]        # [128, kg]
                    flat = blk.T.reshape(-1)   # slot-major: j*128 + d
                    segs.append(_wrap(flat.astype(np.int16)))
                if pas == "lo":
                    lo_pos = np.full(N, cfg.NPC, np.int64)
                    lo_pos[real] = np.arange(len(real))
                    per_core[c]["lo_pos"] = lo_pos
                else:
                    new_pos[real] = np.arange(len(real))
        return new_pos

    hi1 = do_layer(1, tid1)
    tid2 = node_core * SLAB + hi1
    hi2 = do_layer(2, tid2)

    # per-core x slab (layer-1 order), pool indicator (layer-2 hi order)
    cnts = np.bincount(batch.astype(np.int64), minlength=N_GRAPHS).astype(np.float32)
    for c in range(NC):
        nodes_c = np.where(node_core == c)[0]
        xT = np.zeros((F_IN, NPC), np.float32)
        xT[:, pos1[nodes_c]] = x[nodes_c].T
        per_core[c]["xT"] = xT
        bt = np.zeros((NPC, N_GRAPHS), np.float32)
        bt[hi2[nodes_c], batch[nodes_c].astype(np.int64)] = 1.0
        per_core[c]["bt"] = bt
        per_core[c]["seg_l1"] = per_core[c]["idx_l1"]
        per_core[c]["seg_l2"] = per_core[c]["idx_l2"]
        per_core[c]["idx_l1"] = np.concatenate(per_core[c]["idx_l1"], axis=1)
        per_core[c]["idx_l2"] = np.concatenate(per_core[c]["idx_l2"], axis=1)
        del per_core[c]["lo_pos"]
    meta["invc"] = (1.0 / np.maximum(cnts, 1.0)).reshape(N_GRAPHS, 1)
    meta["S1"] = per_core[0]["idx_l1"].shape[1]
    meta["S2"] = per_core[0]["idx_l2"].shape[1]
    meta["node_core"] = node_core
    meta["hi1"] = hi1
    meta["hi2"] = hi2
    return meta, per_core


# ---------------------------------------------------------------------------

def build_kernel(cfg, meta, debug=False):
    nc = bacc.Bacc(None, target_bir_lowering=False, num_swdge_queues=4)
    NPC, SLAB, V, G = cfg.NPC, cfg.SLAB, cfg.V, cfg.G
    if debug:
        h1_dbg = nc.dram_tensor("h1_dbg", [P, NPC], f32, kind="ExternalOutput")
        h2_dbg = nc.dram_tensor("h2_dbg", [P, NPC], f32, kind="ExternalOutput")

    xT_d = nc.dram_tensor("xT", [F_IN, NPC], f32, kind="ExternalInput")
    W1_d = nc.dram_tensor("W1", [F_IN, F], f32, kind="ExternalInput")
    asad1_d = nc.dram_tensor("asad1", [F, 2], f32, kind="ExternalInput")
    b1b_d = nc.dram_tensor("b1b", [P, F], f32, kind="ExternalInput")
    W2_d = nc.dram_tensor("W2", [F, F], f32, kind="ExternalInput")
    asad2_d = nc.dram_tensor("asad2", [F, 2], f32, kind="ExternalInput")
    b2b_d = nc.dram_tensor("b2b", [P, F], f32, kind="ExternalInput")
    fc1w_d = nc.dram_tensor("fc1w", [F, NHID], f32, kind="ExternalInput")
    fc1b_d = nc.dram_tensor("fc1b", [NHID, 1], f32, kind="ExternalInput")
    fc2w_d = nc.dram_tensor("fc2w", [NHID, N_CLS], f32, kind="ExternalInput")
    fc2b_d = nc.dram_tensor("fc2b", [N_CLS, 1], f32, kind="ExternalInput")
    bt_d = nc.dram_tensor("bt", [NPC, N_GRAPHS], f32, kind="ExternalInput")
    invc_d = nc.dram_tensor("invc", [N_GRAPHS, 1], f32, kind="ExternalInput")
    idx1_d = nc.dram_tensor("idx1", [P, meta["S1"]], i16, kind="ExternalInput")
    idx2_d = nc.dram_tensor("idx2", [P, meta["S2"]], i16, kind="ExternalInput")
    out_d = nc.dram_tensor("out", [N_GRAPHS, N_CLS], f32, kind="ExternalOutput")

    qctr = [0]

    def next_q():
        q = qctr[0] % 4
        qctr[0] += 1
        return q

    with tile.TileContext(nc) as tc:
        with (
            tc.tile_pool(name="const", bufs=1) as cpool,
            tc.tile_pool(name="hbuf", bufs=1) as hpool,
            tc.tile_pool(name="dram", bufs=1, space="DRAM") as dpool,
            tc.tile_pool(name="dramsh", bufs=1, space="DRAM") as dspool,
        ):
            ident16 = cpool.tile([P, P], fp16)
            make_identity(nc, ident16[:])
            identf = cpool.tile([P, P], f32)
            make_identity(nc, identf[:])
            idx1_sb = cpool.tile([P, meta["S1"]], i16)
            nc.sync.dma_start(out=idx1_sb[:], in_=idx1_d[:])
            idx2_sb = cpool.tile([P, meta["S2"]], i16)
            nc.sync.dma_start(out=idx2_sb[:], in_=idx2_d[:])
            asad1 = cpool.tile([F, 2], f32)
            nc.sync.dma_start(out=asad1[:], in_=asad1_d[:])
            asad2 = cpool.tile([F, 2], f32)
            nc.sync.dma_start(out=asad2[:], in_=asad2_d[:])
            b1b = cpool.tile([P, F], f32)
            nc.sync.dma_start(out=b1b[:], in_=b1b_d[:])
            b2b = cpool.tile([P, F], f32)
            nc.sync.dma_start(out=b2b[:], in_=b2b_d[:])
            W2sb = cpool.tile([F, F], f32)
            nc.sync.dma_start(out=W2sb[:], in_=W2_d[:])
            zero32 = cpool.tile([32, ROW], bf16)
            nc.vector.memset(zero32[:], 0.0)
            zerof32 = cpool.tile([2, PART_ROW], f32)
            nc.vector.memset(zerof32[:], 0.0)

            h1_sb = hpool.tile([P, NPC], f32)
            h2_sb = hpool.tile([P, NPC], f32)

            slab1 = dpool.tile([SLAB, ROW], bf16)
            slab2 = dpool.tile([SLAB, ROW], bf16)
            part1 = dpool.tile([SLAB + 1, PART_ROW], f32)
            part2 = dpool.tile([SLAB + 1, PART_ROW], f32)
            table1 = dspool.tile([V, ROW], bf16, addr_space="Shared")
            table2 = dspool.tile([V, ROW], bf16, addr_space="Shared")
            pool_in = dpool.tile([N_GRAPHS, F], f32)
            pool_out = dspool.tile([N_GRAPHS, F], f32, addr_space="Shared")

            # zero the dummy partial row referenced by padded hi-pass slots
            nc.sync.dma_start(out=part1[NPC : NPC + 1, :], in_=zerof32[:1, :])
            nc.sync.dma_start(out=part2[NPC : NPC + 1, :], in_=zerof32[:1, :])

            # ---------------- front-end (both layers) ----------------
            def front_end(layer):
                slab = slab1 if layer == 1 else slab2
                asad = asad1 if layer == 1 else asad2
                n_tiles = NPC // 512 if NPC % 512 == 0 else math.ceil(NPC / 512)
                with (
                    tc.tile_pool(name=f"fe{layer}", bufs=2) as fpool,
                    tc.tile_pool(name=f"fep{layer}", bufs=2, space="PSUM") as fpp,
                    tc.tile_pool(name=f"fet{layer}", bufs=1, space="PSUM") as ftp,
                ):
                    if layer == 1:
                        W1sb = fpool.tile([P, F_IN // P, F], f32, name="W1sb", bufs=1)
                        nc.sync.dma_start(
                            out=W1sb[:],
                            in_=W1_d[:].rearrange("(k p) f -> p k f", p=P))
                    for t in range(n_tiles):
                        c0 = t * 512
                        cw = min(512, NPC - c0)
                        xw_ps = fpp.tile([P, 512], f32, tag="xwps")
                        if layer == 1:
                            xt_t = fpool.tile([P, F_IN // P, 512], f32, tag="xt")
                            nc.sync.dma_start(
                                out=xt_t[:, :, :cw],
                                in_=xT_d[:, c0 : c0 + cw].rearrange(
                                    "(k p) n -> p k n", p=P))
                            for k in range(F_IN // P):
                                nc.tensor.matmul(
                                    out=xw_ps[:, :cw],
                                    lhsT=W1sb[:, k, :],
                                    rhs=xt_t[:, k, :cw],
                                    start=(k == 0), stop=(k == F_IN // P - 1))
                        else:
                            hT_t = fpool.tile([P, 512], f32, tag="ht")
                            for b in range(cw // P):
                                hT_ps = ftp.tile([P, P], f32, tag="htps")
                                nc.tensor.transpose(
                                    out=hT_ps[:],
                                    in_=h1_sb[:, c0 + b * P : c0 + (b + 1) * P],
                                    identity=identf[:])
                                nc.scalar.activation(
                                    out=hT_t[:, b * P : (b + 1) * P],
                                    in_=hT_ps[:],
                                    func=mybir.ActivationFunctionType.Copy)
                            nc.tensor.matmul(
                                out=xw_ps[:, :cw], lhsT=W2sb[:],
                                rhs=hT_t[:, :cw], start=True, stop=True)
                        xw_sb = fpool.tile([P, 512], f32, tag="xwsb")
                        nc.scalar.activation(
                            out=xw_sb[:, :cw], in_=xw_ps[:, :cw],
                            func=mybir.ActivationFunctionType.Copy)
                        es_ps = ftp.tile([2, 512], f32, tag="esps")
                        nc.tensor.matmul(out=es_ps[:, :cw], lhsT=asad[:],
                                         rhs=xw_sb[:, :cw], start=True, stop=True)
                        es_sb = fpool.tile([2, 512], f32, tag="essb")
                        nc.vector.tensor_copy(out=es_sb[:, :cw], in_=es_ps[:, :cw])
                        for b in range(cw // P):
                            stg = fpool.tile([P, ROW], bf16, tag="stg")
                            xwT_ps = ftp.tile([P, P], f32, tag="xwtps")
                            nc.tensor.transpose(
                                out=xwT_ps[:],
                                in_=xw_sb[:, b * P : (b + 1) * P],
                                identity=identf[:])
                            nc.vector.memset(stg[:, C_ONE : C_ONE + 1], 1.0)
                            nc.vector.memset(stg[:, 251:252], 0.0)
                            nc.scalar.activation(
                                out=stg[:, C_HI : C_HI + F], in_=xwT_ps[:],
                                func=mybir.ActivationFunctionType.Copy)
                            nc.vector.tensor_tensor(
                                out=stg[:, C_LO : C_LO + FLO],
                                in0=xwT_ps[:, :FLO],
                                in1=stg[:, C_HI : C_HI + FLO],
                                op=mybir.AluOpType.subtract)
                            esT_ps = ftp.tile([P, 2], f32, tag="estps")
                            nc.tensor.transpose(
                                out=esT_ps[:],
                                in_=es_sb[:, b * P : (b + 1) * P],
                                identity=identf[:2, :2])
                            stg_f = stg[:].bitcast(f32)
                            nc.vector.tensor_copy(
                                out=stg_f[:, 126:128], in_=esT_ps[:])
                            nc.sync.dma_start(
                                out=slab[c0 + b * P : c0 + (b + 1) * P, :],
                                in_=stg[:])
                    # zero pad rows of the slab
                    nc.sync.dma_start(
                        out=slab[NPC:SLAB, :], in_=zero32[: SLAB - NPC, :])

            # ---------------- gather pass ----------------
            def gat_pass(layer, pas, idx_sb, off):
                """off: running col offset into idx_sb; returns new offset."""
                table = table1 if layer == 1 else table2
                slab = slab1 if layer == 1 else slab2
                part = part1 if layer == 1 else part2
                bbias = b1b if layer == 1 else b2b
                h_out = h1_sb if layer == 1 else h2_sb
                ks = meta["k"][(layer, pas)]
                kmax = int(max(ks))
                with (
                    tc.tile_pool(name=f"g{layer}{pas}", bufs=2) as gpool,
                    tc.tile_pool(name=f"s{layer}{pas}", bufs=3) as spool,
                    tc.tile_pool(name=f"p{layer}{pas}", bufs=3, space="PSUM") as ppool,
                ):
                    for g in range(G):
                        kg = int(ks[g])
                        # ed (lo) / partial (hi) gather: 128 idxs = 8 cols
                        if pas == "lo":
                            D_t = gpool.tile([P, 1, ROW], bf16, tag="D")
                            nc.gpsimd.dma_gather(
                                out_ap=D_t[:], in_ap=slab[:],
                                idxs_ap=idx_sb[:, off : off + 8],
                                num_idxs=P, num_idxs_reg=P, elem_size=ROW,
                                single_packet=False, queue_num=next_q())
                            ed_col = D_t[:].bitcast(f32)[:, 0, 127:128]
                        else:
                            Pt_t = gpool.tile([P, 1, PART_ROW], f32, tag="Pt")
                            nc.gpsimd.dma_gather(
                                out_ap=Pt_t[:], in_ap=part[:],
                                idxs_ap=idx_sb[:, off : off + 8],
                                num_idxs=P, num_idxs_reg=P, elem_size=PART_ROW,
                                single_packet=False, queue_num=next_q())
                            ed_col = Pt_t[:, 0, 129:130]
                        off += 8
                        # main gather
                        G_t = gpool.tile([P, kmax, ROW], bf16, tag="G")
                        view = table[: cfg.LO, :] if pas == "lo" else table[cfg.LO :, :]
                        nc.gpsimd.dma_gather(
                            out_ap=G_t[:, :kg, :], in_ap=view,
                            idxs_ap=idx_sb[:, off : off + kg * 8],
                            num_idxs=P * kg, num_idxs_reg=P * kg, elem_size=ROW,
                            single_packet=(P * kg <= 1024), queue_num=next_q())
                        off += kg * 8
                        # p = exp(lrelu(es + ed))
                        es_v = G_t[:].bitcast(f32)[:, :kg, 126]
                        s_t = spool.tile([P, kmax], f32, tag="s")
                        nc.vector.tensor_scalar(
                            out=s_t[:, :kg], in0=es_v, scalar1=ed_col,
                            scalar2=None, op0=mybir.AluOpType.add)
                        l_t = spool.tile([P, kmax], f32, tag="l")
                        nc.vector.tensor_scalar(
                            out=l_t[:, :kg], in0=s_t[:, :kg], scalar1=0.2,
                            scalar2=None, op0=mybir.AluOpType.mult)
                        nc.vector.tensor_tensor(
                            out=l_t[:, :kg], in0=l_t[:, :kg], in1=s_t[:, :kg],
                            op=mybir.AluOpType.max)
                        p_t = spool.tile([P, kmax], f32, tag="p")
                        nc.scalar.activation(
                            out=p_t[:, :kg], in_=l_t[:, :kg],
                            func=mybir.ActivationFunctionType.Exp)
                        # chunks: scale + matmul accumulate
                        ps = ppool.tile([P, C_SCALE_END], f32, tag="ps")
                        for j in range(kg):
                            sc = spool.tile([P, C_SCALE_END], fp16, tag="sc")
                            nc.vector.tensor_scalar(
                                out=sc[:], in0=G_t[:, j, :C_SCALE_END],
                                scalar1=p_t[:, j : j + 1], scalar2=None,
                                op0=mybir.AluOpType.mult)
                            nc.tensor.matmul(
                                out=ps[:], lhsT=ident16[:], rhs=sc[:],
                                start=(j == 0), stop=(j == kg - 1))
                        # fold: msg[f] = ps[1+f] + ps[129+f] (f<FLO)
                        if pas == "lo":
                            stg = spool.tile([P, PART_ROW], f32, tag="stg")
                            nc.vector.tensor_copy(
                                out=stg[:, 0 : 1 + F], in_=ps[:, 0 : 1 + F])
                            nc.vector.tensor_tensor(
                                out=stg[:, 1 : 1 + FLO], in0=stg[:, 1 : 1 + FLO],
                                in1=ps[:, C_LO:C_SCALE_END],
                                op=mybir.AluOpType.add)
                            nc.vector.tensor_copy(
                                out=stg[:, 129:130], in_=ed_col)
                            nc.sync.dma_start(
                                out=part[g * P : (g + 1) * P, :], in_=stg[:])
                        else:
                            ta = spool.tile([P, F], f32, tag="ta")
                            nc.vector.tensor_copy(
                                out=ta[:], in_=ps[:, 1 : 1 + F])
                            nc.vector.tensor_tensor(
                                out=ta[:, :FLO], in0=ta[:, :FLO],
                                in1=ps[:, C_LO:C_SCALE_END],
                                op=mybir.AluOpType.add)
                            den = spool.tile([P, 1], f32, tag="den")
                            nc.vector.tensor_tensor(
                                out=den[:], in0=ps[:, 0:1], in1=Pt_t[:, 0, 0:1],
                                op=mybir.AluOpType.add)
                            nc.vector.tensor_scalar(
                                out=den[:], in0=den[:], scalar1=1e-20,
                                scalar2=None, op0=mybir.AluOpType.max)
                            r_t = spool.tile([P, 1], f32, tag="r")
                            nc.vector.reciprocal(out=r_t[:], in_=den[:])
                            tb = spool.tile([P, F], f32, tag="tb")
                            nc.vector.tensor_tensor(
                                out=tb[:], in0=ta[:], in1=Pt_t[:, 0, 1 : 1 + F],
                                op=mybir.AluOpType.add)
                            nc.vector.tensor_scalar(
                                out=tb[:], in0=tb[:], scalar1=r_t[:],
                                scalar2=None, op0=mybir.AluOpType.mult)
                            nc.vector.tensor_tensor(
                                out=tb[:], in0=tb[:], in1=bbias[:],
                                op=mybir.AluOpType.add)
                            _selu(tb, h_out[:, g * P : (g + 1) * P], spool)
                return off

            def _selu(x_t, out_ap, spool):
                mn = spool.tile([P, F], f32, tag="selmn")
                nc.vector.tensor_scalar(
                    out=mn[:], in0=x_t[:], scalar1=0.0, scalar2=None,
                    op0=mybir.AluOpType.min)
                ex = spool.tile([P, F], f32, tag="selex")
                nc.scalar.activation(
                    out=ex[:], in_=mn[:], func=mybir.ActivationFunctionType.Exp)
                nc.vector.tensor_scalar(
                    out=ex[:], in0=ex[:], scalar1=SELU_L * SELU_A,
                    scalar2=-SELU_L * SELU_A, op0=mybir.AluOpType.mult,
                    op1=mybir.AluOpType.add)
                mx = spool.tile([P, F], f32, tag="selmx")
                nc.vector.tensor_scalar(
                    out=mx[:], in0=x_t[:], scalar1=0.0, scalar2=SELU_L,
                    op0=mybir.AluOpType.max, op1=mybir.AluOpType.mult)
                nc.vector.tensor_tensor(
                    out=out_ap, in0=mx[:], in1=ex[:], op=mybir.AluOpType.add)

            # ---------------- run it ----------------
            front_end(1)
            nc.gpsimd.collective_compute(
                "AllGather", mybir.AluOpType.bypass,
                ins=[slab1[:].opt()], outs=[table1[:].opt()],
                replica_groups=[list(range(NC))])
            off = gat_pass(1, "lo", idx1_sb, 0)
            off = gat_pass(1, "hi", idx1_sb, off)
            assert off == meta["S1"], (off, meta["S1"])

            front_end(2)
            nc.gpsimd.collective_compute(
                "AllGather", mybir.AluOpType.bypass,
                ins=[slab2[:].opt()], outs=[table2[:].opt()],
                replica_groups=[list(range(NC))])
            off = gat_pass(2, "lo", idx2_sb, 0)
            off = gat_pass(2, "hi", idx2_sb, off)
            assert off == meta["S2"], (off, meta["S2"])
            if debug:
                nc.sync.dma_start(out=h1_dbg[:], in_=h1_sb[:])
                nc.sync.dma_start(out=h2_dbg[:], in_=h2_sb[:])

            # ---------------- pooling + head ----------------
            with (
                tc.tile_pool(name="head", bufs=2) as hp,
                tc.tile_pool(name="headp", bufs=2, space="PSUM") as hpp,
            ):
                pool_ps = hpp.tile([N_GRAPHS, F], f32, name="poolps", bufs=1)
                for g in range(G):
                    btg = hp.tile([P, N_GRAPHS], f32, tag="btg")
                    nc.sync.dma_start(
                        out=btg[:], in_=bt_d[g * P : (g + 1) * P, :])
                    nc.tensor.matmul(
                        out=pool_ps[:], lhsT=btg[:],
                        rhs=h2_sb[:, g * P : (g + 1) * P],
                        start=(g == 0), stop=(g == G - 1))
                psum_sb = hp.tile([N_GRAPHS, F], f32, name="psum_sb", bufs=1)
                nc.vector.tensor_copy(out=psum_sb[:], in_=pool_ps[:])
                nc.sync.dma_start(out=pool_in[:], in_=psum_sb[:])
                nc.gpsimd.collective_compute(
                    "AllReduce", mybir.AluOpType.add,
                    ins=[pool_in[:].opt()], outs=[pool_out[:].opt()],
                    replica_groups=[list(range(NC))])
                pooled = hp.tile([N_GRAPHS, F], f32, name="pooled", bufs=1)
                nc.sync.dma_start(out=pooled[:], in_=pool_out[:])
                invc = hp.tile([N_GRAPHS, 1], f32, name="invc", bufs=1)
                nc.sync.dma_start(out=invc[:], in_=invc_d[:])
                nc.vector.tensor_scalar(
                    out=pooled[:], in0=pooled[:], scalar1=invc[:],
                    scalar2=None, op0=mybir.AluOpType.mult)
                with tc.tile_pool(name="selp", bufs=1) as sp2:
                    _selu(pooled, pooled[:], sp2)
                    # fc1: need pooled.T
                    pT_ps = hpp.tile([F, N_GRAPHS], f32, name="ptps", bufs=1)
                    nc.tensor.transpose(
                        out=pT_ps[:], in_=pooled[:], identity=identf[:])
                    pT = hp.tile([F, N_GRAPHS], f32, name="pT", bufs=1)
                    nc.vector.tensor_copy(out=pT[:], in_=pT_ps[:])
                    fc1w = hp.tile([F, NHID], f32, name="fc1w", bufs=1)
                    nc.sync.dma_start(out=fc1w[:], in_=fc1w_d[:])
                    g1_ps = hpp.tile([NHID, N_GRAPHS], f32, name="g1ps", bufs=1)
                    nc.tensor.matmul(out=g1_ps[:], lhsT=fc1w[:], rhs=pT[:],
                                     start=True, stop=True)
                    g1 = hp.tile([NHID, N_GRAPHS], f32, name="g1", bufs=1)
                    fc1b = hp.tile([NHID, 1], f32, name="fc1b", bufs=1)
                    nc.sync.dma_start(out=fc1b[:], in_=fc1b_d[:])
                    nc.vector.tensor_scalar(
                        out=g1[:], in0=g1_ps[:], scalar1=fc1b[:],
                        scalar2=None, op0=mybir.AluOpType.add)
                    # selu on [64, 128]
                    mn = hp.tile([NHID, N_GRAPHS], f32, name="selmn2", bufs=1)
                    nc.vector.tensor_scalar(
                        out=mn[:], in0=g1[:], scalar1=0.0, scalar2=None,
                        op0=mybir.AluOpType.min)
                    nc.scalar.activation(
                        out=mn[:], in_=mn[:],
                        func=mybir.ActivationFunctionType.Exp)
                    nc.vector.tensor_scalar(
                        out=mn[:], in0=mn[:], scalar1=SELU_L * SELU_A,
                        scalar2=-SELU_L * SELU_A, op0=mybir.AluOpType.mult,
                        op1=mybir.AluOpType.add)
                    nc.vector.tensor_scalar(
                        out=g1[:], in0=g1[:], scalar1=0.0, scalar2=SELU_L,
                        op0=mybir.AluOpType.max, op1=mybir.AluOpType.mult)
                    nc.vector.tensor_tensor(
                        out=g1[:], in0=g1[:], in1=mn[:],
                        op=mybir.AluOpType.add)
                    fc2w = hp.tile([NHID, N_CLS], f32, name="fc2w", bufs=1)
                    nc.sync.dma_start(out=fc2w[:], in_=fc2w_d[:])
                    lg_ps = hpp.tile([N_CLS, N_GRAPHS], f32, name="lgps", bufs=1)
                    nc.tensor.matmul(out=lg_ps[:], lhsT=fc2w[:], rhs=g1[:],
                                     start=True, stop=True)
                    lg = hp.tile([N_CLS, N_GRAPHS], f32, name="lg", bufs=1)
                    fc2b = hp.tile([N_CLS, 1], f32, name="fc2b", bufs=1)
                    nc.sync.dma_start(out=fc2b[:], in_=fc2b_d[:])
                    nc.vector.tensor_scalar(
                        out=lg[:], in0=lg_ps[:], scalar1=fc2b[:],
                        scalar2=None, op0=mybir.AluOpType.add)
                    lgT_ps = hpp.tile([N_GRAPHS, N_CLS], f32, name="lgtps", bufs=1)
                    nc.tensor.transpose(
                        out=lgT_ps[:], in_=lg[:], identity=identf[:N_CLS, :N_CLS])
                    lgT = hp.tile([N_GRAPHS, N_CLS], f32, name="lgT", bufs=1)
                    nc.vector.tensor_copy(out=lgT[:], in_=lgT_ps[:])
                    # log_softmax over the 2 classes (free dim)
                    m = hp.tile([N_GRAPHS, 1], f32, name="lsm", bufs=1)
                    nc.vector.tensor_reduce(
                        out=m[:], in_=lgT[:], axis=mybir.AxisListType.X,
                        op=mybir.AluOpType.max)
                    nm = hp.tile([N_GRAPHS, 1], f32, name="lsnm", bufs=1)
                    nc.vector.tensor_scalar(
                        out=nm[:], in0=m[:], scalar1=-1.0, scalar2=None,
                        op0=mybir.AluOpType.mult)
                    e = hp.tile([N_GRAPHS, N_CLS], f32, name="lse", bufs=1)
                    nc.scalar.activation(
                        out=e[:], in_=lgT[:],
                        func=mybir.ActivationFunctionType.Exp, bias=nm[:])
                    se = hp.tile([N_GRAPHS, 1], f32, name="lsse", bufs=1)
                    nc.vector.tensor_reduce(
                        out=se[:], in_=e[:], axis=mybir.AxisListType.X,
                        op=mybir.AluOpType.add)
                    nc.scalar.activation(
                        out=se[:], in_=se[:],
                        func=mybir.ActivationFunctionType.Ln)
                    sh = hp.tile([N_GRAPHS, 1], f32, name="lssh", bufs=1)
                    nc.vector.tensor_tensor(
                        out=sh[:], in0=m[:], in1=se[:], op=mybir.AluOpType.add)
                    res = hp.tile([N_GRAPHS, N_CLS], f32, name="res", bufs=1)
                    nc.vector.tensor_scalar(
                        out=res[:], in0=lgT[:], scalar1=sh[:], scalar2=None,
                        op0=mybir.AluOpType.subtract)
                    nc.sync.dma_start(out=out_d[:], in_=res[:])

    nc.finalize()
    return nc


# ---------------------------------------------------------------------------

def run_gat(cfg, x, edge_index, batch, W1, as1, ad1, b1, W2, as2, ad2, b2,
            fc1_w, fc1_b, fc2_w, fc2_b, trace=False, debug=False):
    x = np.asarray(x, np.float32)
    meta, per_core = build_host(cfg, x, np.asarray(edge_index),
                                np.asarray(batch))
    nc = build_kernel(cfg, meta, debug=debug)
    shared = {
        "W1": np.asarray(W1, np.float32),
        "asad1": np.stack([np.asarray(as1, np.float32),
                           np.asarray(ad1, np.float32)], axis=1),
        "b1b": np.tile(np.asarray(b1, np.float32)[None, :], (P, 1)),
        "W2": np.asarray(W2, np.float32),
        "asad2": np.stack([np.asarray(as2, np.float32),
                           np.asarray(ad2, np.float32)], axis=1),
        "b2b": np.tile(np.asarray(b2, np.float32)[None, :], (P, 1)),
        "fc1w": np.asarray(fc1_w, np.float32),
        "fc1b": np.asarray(fc1_b, np.float32).reshape(NHID, 1),
        "fc2w": np.asarray(fc2_w, np.float32),
        "fc2b": np.asarray(fc2_b, np.float32).reshape(N_CLS, 1),
        "invc": meta["invc"].astype(np.float32),
    }
    in_maps = []
    for c in range(NC):
        m = dict(shared)
        m["xT"] = per_core[c]["xT"]
        m["bt"] = per_core[c]["bt"]
        m["idx1"] = per_core[c]["idx_l1"]
        m["idx2"] = per_core[c]["idx_l2"]
        in_maps.append(m)
    res = run_bass_kernel_spmd(nc, in_maps, core_ids=list(range(NC)),
                               trace=trace)
    return res.results[0]["out"], res, meta


def kernel(**inputs):
    out = run_gat(
        FULL,
        inputs["x"], inputs["edge_index"], inputs["batch"],
        inputs["W1"], inputs["as1"], inputs["ad1"], inputs["b1"],
        inputs["W2"], inputs["as2"], inputs["ad2"], inputs["b2"],
        inputs["fc1_w"], inputs["fc1_b"], inputs["fc2_w"], inputs["fc2_b"])[0]
    return out
